# revision 1
# baseline (speedup 1.0000x reference)
"""Dynamic depthwise 3x3 conv (per-pixel weights) on 8 Trainium2 NeuronCores.

Problem:
  x:            [4, 64, 256, 256]  f32
  conv_weights: [4, 576, 256, 256] f32  (= [4, 64ch * 9tap, 256, 256])
  out[n,c,h,w] = sum_k w[n, c*9+k, h, w] * xpad[n, c, h+ki, w+kj],  k=(ki,kj) row-major

Sharding: pure data parallel over (batch n, H-half) -> 8 shards.

On-core layout: partition p = hb*64 + c (hb in {0,1} picks a 64-row block of
the core's 128 output rows, c the channel). x is stored UNPADDED in W
(rows of 256, H-padded on the host), so the flattened (h, w) index j is
contiguous and a single custom-DVE segmented-MAC instruction covers a whole
Rh-row tile for one kernel-row dh:

    tgt[p, j] = sum_dw w[p, dh, j, dw] * x[p, j + dh*256 + dw - 1]

Width-edge taps (wd=0,dw=0 and wd=255,dw=2) would wrap into the neighbouring
row; in the reference those taps multiply zero-padding, so the host repack
zeroes those weight entries and the wrap contributions vanish exactly.

conv_weights are repacked host-side to [T, 128, dh, (h,w), dw] so each
DMA is one sequential HBM stream (~27 GB/s/engine descriptors) and the MAC's
dw-segments are innermost. The custom DVE op (segmac.py) does the 3-tap
dot product per output element in one pass with a per-segment scan reset.
"""

import sys

sys.path.insert(0, "/opt/trn_rl_repo")

import numpy as np

import concourse.bass as bass
import concourse.bacc as bacc
import concourse.tile as tile
from concourse import mybir
from concourse.bass_utils import run_bass_kernel_spmd


# ---------------------------------------------------------------------------
# Custom DVE op: segmented multiply-accumulate (dot-KW per output element).
#   out[p, s] = sum_n in0[p, s, n] * in1[p, s, n]
# A scan(ADD, Src0*Src1) whose accumulator resets at each SUB_DIM_DONE (the
# per-page "per_subdim" STEP state the HW supports but the stock Spec DSL does
# not expose); the out AP uses a step-0 inner dim so the last (complete)
# partial of each segment is what lands at out[p, s]. Streams both tensors at
# 1 elem/lane/cycle: a 3-tap dot product costs 3 input cycles, no reduction
# passes.
# ---------------------------------------------------------------------------

from dataclasses import dataclass

import concourse.dve_spec as dve_spec
import concourse.dve_ops as dve_ops
from concourse.dve_spec import AluOp, Spec, Src0, Src1
from concourse.dve_uop import DveOpSpec

OP_NAME = "SEG_MAC_ANT"



@dataclass(frozen=True)
class _ResetScan(dve_spec.Scan):
    """scan() that re-seeds from `init` at each SUB_DIM_DONE."""


def _patched_scan_overrides(scans, node_stage):
    seed, step = {}, {}
    for scan in scans:
        d = node_stage[scan]
        init = dve_spec._scan_init(scan)
        seed[d] = dve_spec._node_as_stage(init)
        if isinstance(scan, _ResetScan):
            # Page boundary: restart the fold — d = init op expr (the
            # "per_subdim" STEP variant from the HW state-machine table).
            step[d] = dve_spec._Stage(scan.op, init, scan.expr)
        elif scan._subdim_step is not None:
            step[d] = dve_spec._Stage(
                scan.op, dve_spec.AluInp.CURR_ALU_OUT, scan._subdim_step
            )
    return seed, step


def _segmac_ref(in0, in1, c0, c1, c2):
    # CoreSim reference: per-segment inclusive prefix of the products.
    return np.cumsum(
        np.asarray(in0, np.float32) * np.asarray(in1, np.float32),
        axis=-1,
        dtype=np.float32,
    )


def get_segmac_op():
    """Build + register the op (idempotent). Returns the DveOp."""
    existing = getattr(dve_ops, "_ANT_SEG_MAC", None)
    if existing is not None:
        return existing

    dve_spec._scan_overrides = _patched_scan_overrides

    body = _ResetScan(AluOp.ADD, Src0 * Src1)
    spec = Spec(body=body, reference=_segmac_ref)

    shas = {}
    for ver in ("v3", "v4"):
        uops = dve_spec.lower(spec, ver=ver)
        shas[ver] = DveOpSpec(name=OP_NAME, uops=uops, rd1_en=True).sha(ver)

    op = dve_ops.DveOp(OP_NAME, spec, subdim=True, uops_sha=shas)
    dve_ops.OPS.append(op)
    dve_ops._SUB_OPCODE_FOR_NAME[OP_NAME] = (
        dve_ops._CUSTOM_DVE_ROW_BASE + len(dve_ops.OPS) - 1
    )
    dve_ops.CUSTOM_DVE_SPECS[OP_NAME] = spec
    assert dve_ops._SUB_OPCODE_FOR_NAME[OP_NAME] < 0x20
    dve_ops._ANT_SEG_MAC = op
    return op


def window_ap(sl, dims):
    """Build an AP over `sl`'s tensor/offset with explicit free dims
    [[step, count], ...] (partition dim copied from sl)."""
    import bass_rust

    return bass_rust.AP(
        sl.tensor,
        sl.offset,
        [list(sl.ap[0])] + [list(d) for d in dims],
        sl.const_val,
        sl.runtime_checks,
        sl.dep_tracking_offset,
    )


N, C, H, W = 4, 64, 256, 256
KW = 3
NCORES = 8
HH = H // 2          # rows per core
RB = HH // 2         # rows per partition block (64)
Rh = 4               # rows per h-tile
T = RB // Rh         # h-tiles per core
NXT = 4              # resident x tiles per core
XB = RB // NXT       # local output rows covered per x tile (16)
XR = XB + 2          # rows per resident x tile incl halo
XF = XR * W + 2      # x tile free elems incl 1 zero guard at each end
J = Rh * W           # flattened (h, w) positions per tile
WF = KW * KW * J     # w tile free elems
F32 = mybir.dt.float32

_CACHE = {}


def _build():
    op = get_segmac_op()
    nc = bacc.Bacc("TRN2", target_bir_lowering=False, debug=False, num_devices=NCORES)
    x_in = nc.dram_tensor("x", [NXT, 128, XF], F32, kind="ExternalInput")
    w_in = nc.dram_tensor("w", [T, 128, WF], F32, kind="ExternalInput")
    y_out = nc.dram_tensor("y", [T, 128, J], F32, kind="ExternalOutput")

    with tile.TileContext(nc) as tc:
        with (
            tc.tile_pool(name="xp", bufs=1) as xpool,
            tc.tile_pool(name="wp", bufs=2) as wpool,
            tc.tile_pool(name="op", bufs=3) as opool,
            tc.tile_pool(name="pa", bufs=1) as papool,
            tc.tile_pool(name="pb", bufs=1) as pbpool,
        ):
            # x stays resident: NXT tiles, each covering XB output rows
            # (+2 halo rows) per partition block, loaded once.
            xtiles = []
            for s in range(NXT):
                xt = xpool.tile([128, XF], F32, tag=f"x{s}")
                nc.scalar.dma_start(out=xt[:], in_=x_in[s])
                xtiles.append(xt)

            for t in range(T):
                wt = wpool.tile([128, WF], F32)
                # 3 chunked loads (one per dh group) so the first MAC can
                # start before the whole tile lands.
                for dh in range(KW):
                    c0 = dh * KW * J
                    nc.sync.dma_start(
                        out=wt[:, c0:c0 + KW * J],
                        in_=w_in[t, :, c0:c0 + KW * J],
                    )

                xt = xtiles[t * Rh // XB]
                rbase = t * Rh - (t * Rh // XB) * XB

                ot = opool.tile([128, J], F32)
                pa = papool.tile([128, J], F32)
                pb = pbpool.tile([128, J], F32)
                # one whole-tile segmented MAC per kernel row dh:
                #   tgt[p, j] = sum_dw w[dh, j, dw] * x[(rbase+dh)*W + j + dw - 1]
                # (x AP offset: the +1 guard shift and -1 dw base cancel)
                for dh, tgt in ((0, ot), (1, pa), (2, pb)):
                    w_sl = wt[:, dh * KW * J:(dh + 1) * KW * J]
                    x_sl = xt[:, (rbase + dh) * W:(rbase + dh) * W + J + 2]
                    nc.vector._custom_dve(
                        op,
                        out=window_ap(tgt[:, 0:J], [[1, J], [0, KW]]),
                        in0=window_ap(w_sl, [[KW, J], [1, KW]]),
                        in1=window_ap(x_sl, [[1, J], [1, KW]]),
                    )
                nc.vector.tensor_add(ot[:], ot[:], pa[:])
                nc.vector.tensor_add(ot[:], ot[:], pb[:])

                nc.scalar.dma_start(out=y_out[t], in_=ot[:])
    nc.compile()
    return nc


def _get_nc():
    if "nc" not in _CACHE:
        _CACHE["nc"] = _build()
    return _CACHE["nc"]


def _pack_core(xh_n: np.ndarray, w5_n: np.ndarray, hf: int):
    """Repack one core's shard into per-tile-contiguous DMA blocks.

    xh_n: [C, H+2, W] H-padded x for batch n; w5_n: [C, 9, H, W].
    Returns x_blocks [NXT, 128, XF], w_blocks [T, 128, WF].
    """
    xc = xh_n[:, hf * HH:hf * HH + HH + 2, :]          # [C, HH+2, W]
    wc = w5_n[:, :, hf * HH:(hf + 1) * HH, :]          # [C, 9, HH, W]

    xb = np.zeros((NXT, 2, C, XR * W + 2), dtype=np.float32)
    for s in range(NXT):
        for hb in range(2):
            r0 = hb * RB + s * XB
            xb[s, hb, :, 1:-1] = xc[:, r0:r0 + XR, :].reshape(C, XR * W)
    # w: [C, (dh, dw), (hb, t, r), wd] -> [t, (hb, c), dh, (r, wd), dw]
    wb = (
        wc.reshape(C, KW, KW, 2, T, Rh, W)
        .transpose(4, 3, 0, 1, 5, 6, 2)
        .copy()
    )  # [T, hb, C, dh, r, wd, dw]
    # width-edge taps multiply zero padding in the reference -> zero them
    wb[:, :, :, :, :, 0, 0] = 0.0
    wb[:, :, :, :, :, W - 1, KW - 1] = 0.0
    return (
        xb.reshape(NXT, 128, XF),
        np.ascontiguousarray(wb.reshape(T, 128, WF)),
    )


def _make_in_maps(x: np.ndarray, conv_weights: np.ndarray):
    x = np.asarray(x, dtype=np.float32)
    w5 = np.asarray(conv_weights, dtype=np.float32).reshape(N, C, KW * KW, H, W)
    xh = np.pad(x, ((0, 0), (0, 0), (1, 1), (0, 0)))

    in_maps = []
    for i in range(NCORES):
        n, hf = divmod(i, 2)
        xb, wb = _pack_core(xh[n], w5[n], hf)
        in_maps.append({"x": xb, "w": wb})
    return in_maps


def kernel(x: np.ndarray, conv_weights: np.ndarray) -> np.ndarray:
    nc = _get_nc()
    in_maps = _make_in_maps(x, conv_weights)
    res = run_bass_kernel_spmd(nc, in_maps, list(range(NCORES)))
    out = np.empty((N, C, H, W), dtype=np.float32)
    for i in range(NCORES):
        n, hf = divmod(i, 2)
        yb = res.results[i]["y"].reshape(T, 2, C, Rh, W)
        # invert: out rows h = hf*HH + hb*RB + t*Rh + h_sub
        oc = yb.transpose(2, 1, 0, 3, 4).reshape(C, HH, W)
        out[n, :, hf * HH:(hf + 1) * HH, :] = oc
    return out



# revision 2
# speedup vs baseline: 1.4496x; 1.4496x over previous
"""Dynamic depthwise 3x3 conv (per-pixel weights) on 8 Trainium2 NeuronCores.

Problem:
  x:            [4, 64, 256, 256]  f32
  conv_weights: [4, 576, 256, 256] f32  (= [4, 64ch * 9tap, 256, 256])
  out[n,c,h,w] = sum_k w[n, c*9+k, h, w] * xpad[n, c, h+ki, w+kj],  k=(ki,kj)

Sharding: pure data parallel over (batch n, H-half) -> 8 shards.

This version moves all streams to bf16 (halves HBM traffic; rel err ~0.5%
vs the 2e-2 gate) and replaces the 1-elem/cycle segmented MAC with a custom
2x-mode DVE op (SEG_MAC2_ANT) that retires 2 bf16 MACs/cycle:

  * outputs are processed in adjacent pairs (j, j+1); each DVE cycle reads a
    packed bf16 pair of weights (one tap for both outputs) and a packed pair
    of x values
  * x is stored in an "overlapped pairs" layout x3[2e]=x[e], x3[2e+1]=x[e+1],
    which turns the overlapping 3-tap windows into contiguous aligned reads
  * two independent accumulators (even/odd j) live on different ALU blocks;
    subdim pages of 6 elements = 3 cycles per 3-tap dot; writes are gated to
    the page-final cycle (out_last_subdim_enable), one packed write per pair
  * the instruction carries perf_max=1 in byte-36[7:6] (the firmware decodes
    it; bass.py never sets it, so a scoped monkeypatch injects it)

Per tile (J=2048 outputs/partition): 3 per-dh segmacs (3 cycles/pair each)
+ 2 bf16 tensor_adds combine the dh partials. Weight-edge taps (wd=0,dw=0 /
wd=255,dw=2) are zeroed host-side so width wrap-around contributes nothing.
"""

import sys

sys.path.insert(0, "/opt/trn_rl_repo")

import numpy as np
import ml_dtypes

import concourse.bass as bass
import concourse.bacc as bacc
import concourse.tile as tile
from concourse import mybir
from concourse.bass_utils import run_bass_kernel_spmd

import concourse.dve_ops as dve_ops
import concourse.bass_isa as bass_isa
from concourse.dve_spec import Spec, Src0, Src1
from concourse.dve_uop import (
    ENABLE,
    AluInp,
    AluOp,
    DelayInp,
    DveOpSpec,
    InpSel,
    OutPath,
    OutSel,
    Trigger,
    UopConfig,
    UopDpConfig,
)

# ---------------------------------------------------------------------------
# SEG_MAC2_ANT: j-paired segmented 3-tap MAC, 2 bf16 MACs/cycle.
#   out[p, 2*jp + q] = sum_dw w[p, 6*jp + 2*dw + q] * x3[p, 4*jp + 2*dw + q]
# ---------------------------------------------------------------------------

OP_NAME = "SEG_MAC2_ANT"

_SRC = Trigger.SRC_TENSOR_DONE
_SUB = Trigger.SUB_DIM_DONE
_CNT = Trigger.COUNT
_NON = Trigger.NONE


def _dp(mode):
    """mode: 'seed' (acc<-0), 'steady' (acc+=p), 'step' (acc=0+p).
    Chains: c0=SRC_0 then p_e, c1=SRC_1 then p_o, c2=SRC_0_HI then acc_e,
    c3=SRC_1_HI, c4=ZERO. acc_e at block b2, acc_o at block b7."""
    dp = [UopDpConfig() for _ in range(8)]
    for st in range(8):
        dp[st].pass_through_delay(0, 1, 2, 3, 4)
    dp[0].enable_alu(AluOp.MULTIPLY, AluInp.PREV_DELAY_0, AluInp.PREV_DELAY_1)
    dp[1].enable_alu(AluOp.MULTIPLY, AluInp.PREV_DELAY_2, AluInp.PREV_DELAY_3)
    dp[1].enable_delay_from_src(DelayInp.PREV_ALU_OUT, 0)  # capture p_e
    if mode == "seed":
        dp[2].enable_alu(AluOp.BYPASS, AluInp.PREV_DELAY_4, AluInp.PREV_DELAY_4)
    elif mode == "steady":
        dp[2].enable_alu(AluOp.ADD, AluInp.CURR_ALU_OUT, AluInp.PREV_DELAY_0)
    else:
        dp[2].enable_alu(AluOp.ADD, AluInp.PREV_DELAY_4, AluInp.PREV_DELAY_0)
    dp[2].enable_delay_from_src(DelayInp.PREV_ALU_OUT, 1)  # capture p_o
    dp[3].enable_delay_from_src(DelayInp.PREV_ALU_OUT, 2)  # capture acc_e
    for st in range(3, 7):
        dp[st].pass_through_alu()
    if mode == "seed":
        dp[7].enable_alu(AluOp.BYPASS, AluInp.PREV_DELAY_4, AluInp.PREV_DELAY_4)
    elif mode == "steady":
        dp[7].enable_alu(AluOp.ADD, AluInp.CURR_ALU_OUT, AluInp.PREV_DELAY_1)
    else:
        dp[7].enable_alu(AluOp.ADD, AluInp.PREV_DELAY_4, AluInp.PREV_DELAY_1)
    return dp


def _uop(mode, trig, nxt, repeat, consume, write):
    u = UopConfig(datapath_config=_dp(mode))
    u.enable_input(InpSel.SRC_0, 1)
    u.enable_input(InpSel.SRC_1, 2)
    u.enable_input(InpSel.SRC_0_HI, 3)
    u.enable_input(InpSel.SRC_1_HI, 4)
    u.enable_input(InpSel.ZERO, 5)
    if write:
        u.enable_output(OutSel.DELAY_2, OutPath.WR0_LO)   # acc_e
        u.enable_output(OutSel.ALU_OUT, OutPath.WR0_HI)   # acc_o (b7)
        u.out_last_subdim_enable = ENABLE
    if consume:
        u.require_inp0 = ENABLE
        u.require_inp1 = ENABLE
    u.repeat_count = repeat
    u.trigger = trig
    u.next_uop = nxt
    return u


def _program():
    return [
        _uop("seed", (_CNT, _NON, _NON), (1, 0, 0), 1, False, False),
        _uop("steady", (_SRC, _SUB, _NON), (0, 2, 0), 0, True, True),
        _uop("step", (_SRC, _SUB, _CNT), (0, 2, 1), 1, True, True),
    ]


def _segmac2_ref(in0, in1, c0, c1, c2):
    P = in0.shape[0]
    a = np.asarray(in0, np.float32).reshape(P, -1, 3, 2)
    b = np.asarray(in1, np.float32).reshape(P, -1, 3, 2)
    return (a * b).sum(axis=2)


def get_segmac2_op():
    existing = getattr(dve_ops, "_ANT_SEG_MAC2", None)
    if existing is not None:
        return existing

    spec = Spec(body=Src0 * Src1, reference=_segmac2_ref)
    op = dve_ops.DveOp(OP_NAME, spec, subdim=True, uops_sha={})
    dve_ops.OPS.append(op)
    row = dve_ops._CUSTOM_DVE_ROW_BASE + len(dve_ops.OPS) - 1
    assert row < 0x20
    dve_ops._SUB_OPCODE_FOR_NAME[OP_NAME] = row
    dve_ops.CUSTOM_DVE_SPECS[OP_NAME] = spec

    compiled = DveOpSpec(
        name=OP_NAME,
        opcode=row,
        uops=_program(),
        uops_2x=_program(),
        rd1_en=True,
        perf_max=1,
    )
    compiled.validate("v3")
    dve_ops._COMPILE_CACHE[(OP_NAME, "v3")] = compiled

    import concourse.bass as bass_mod

    orig = bass_isa.InstCustomDveAnt

    def patched(*args, **kwargs):
        if kwargs.get("op_name") == OP_NAME:
            kwargs.setdefault("perf_max", 1)
        return orig(*args, **kwargs)

    bass_isa.InstCustomDveAnt = patched
    bass_mod.bass_isa.InstCustomDveAnt = patched

    dve_ops._ANT_SEG_MAC2 = op
    return op


def window_ap(sl, dims):
    """AP over `sl`'s tensor/offset with explicit free dims [[step, count],...]."""
    import bass_rust

    return bass_rust.AP(
        sl.tensor,
        sl.offset,
        [list(sl.ap[0])] + [list(d) for d in dims],
        sl.const_val,
        sl.runtime_checks,
        sl.dep_tracking_offset,
    )


# ---------------------------------------------------------------------------
# Kernel
# ---------------------------------------------------------------------------

N, C, H, W = 4, 64, 256, 256
KW = 3
NCORES = 8
HH = H // 2          # rows per core
RB = HH // 2         # rows per partition block (64)
Rh = 8               # rows per h-tile
T = RB // Rh         # h-tiles per core (8)
J = Rh * W           # outputs per partition per tile (2048)
JP = J // 2          # output pairs (1024)
WSEG = 6 * JP        # w elems per dh chunk
WF = 3 * WSEG        # w elems per tile (9*J)
NXT = 4              # resident x tiles per core
XB = RB // NXT       # output rows per x tile (16)
XR = XB + 2          # rows per x tile incl halo (18)
X3F = 2 * XR * W + 4  # doubled x elems per x tile (+guards)
BF = mybir.dt.bfloat16
BF16 = ml_dtypes.bfloat16

_CACHE = {}


def _build():
    op = get_segmac2_op()
    nc = bacc.Bacc("TRN2", target_bir_lowering=False, debug=False, num_devices=NCORES)
    x_in = nc.dram_tensor("x", [NXT, 128, X3F], BF, kind="ExternalInput")
    w_in = nc.dram_tensor("w", [T, 128, WF], BF, kind="ExternalInput")
    y_out = nc.dram_tensor("y", [T, 128, J], BF, kind="ExternalOutput")

    with tile.TileContext(nc) as tc:
        with (
            tc.tile_pool(name="xp", bufs=1) as xpool,
            tc.tile_pool(name="wp", bufs=2) as wpool,
            tc.tile_pool(name="o0", bufs=2) as o0pool,
            tc.tile_pool(name="pa", bufs=1) as papool,
            tc.tile_pool(name="pb", bufs=1) as pbpool,
        ):
            xtiles = []
            for s in range(NXT):
                xt = xpool.tile([128, X3F], BF, tag=f"x{s}")
                nc.scalar.dma_start(out=xt[:], in_=x_in[s])
                xtiles.append(xt)

            for t in range(T):
                wt = wpool.tile([128, WF], BF)
                for dh in range(KW):
                    nc.sync.dma_start(
                        out=wt[:, dh * WSEG:(dh + 1) * WSEG],
                        in_=w_in[t, :, dh * WSEG:(dh + 1) * WSEG],
                    )

                s = t * Rh // XB
                rb = t * Rh - s * XB
                xt = xtiles[s]

                ot = o0pool.tile([128, J], BF)
                pa = papool.tile([128, J], BF)
                pb = pbpool.tile([128, J], BF)
                for dh, tgt in ((0, ot), (1, pa), (2, pb)):
                    xbase = 2 * (rb + dh) * W
                    nc.vector._custom_dve(
                        op,
                        out=window_ap(tgt[:, 0:J], [[2, JP], [1, 2]]),
                        in0=window_ap(
                            wt[:, dh * WSEG:(dh + 1) * WSEG], [[6, JP], [1, 6]]
                        ),
                        in1=window_ap(
                            xt[:, xbase:xbase + 4 * JP + 2], [[4, JP], [1, 6]]
                        ),
                    )
                nc.vector.tensor_add(ot[:], ot[:], pa[:])
                nc.vector.tensor_add(ot[:], ot[:], pb[:])

                nc.scalar.dma_start(out=y_out[t], in_=ot[:])
    nc.compile()
    return nc


def _get_nc():
    if "nc" not in _CACHE:
        _CACHE["nc"] = _build()
    return _CACHE["nc"]


def _pack_core(xh_n: np.ndarray, w5_n: np.ndarray, hf: int):
    """Repack one core's shard (bf16 inputs).

    xh_n: [C, H+2, W] H-padded x for batch n (bf16); w5_n: [C, 9, H, W] bf16.
    Returns x_blocks [NXT, 128, X3F], w_blocks [T, 128, WF].
    """
    xc = xh_n[:, hf * HH:hf * HH + HH + 2, :]          # [C, HH+2, W]
    wc = w5_n[:, :, hf * HH:(hf + 1) * HH, :]          # [C, 9, HH, W]

    # x3: doubled layout per x tile, per partition block
    xb = np.zeros((NXT, 2, C, X3F), dtype=BF16)
    L = XR * W
    for s in range(NXT):
        for hb in range(2):
            r0 = hb * RB + s * XB
            flat = xc[:, r0:r0 + XR, :].reshape(C, L)
            ext = np.zeros((C, L + 2), dtype=BF16)
            ext[:, :L] = flat
            if r0 + XR < HH + 2:
                ext[:, L:L + 2] = xc[:, r0 + XR, :2]
            xb[s, hb, :, 2:2 + 2 * L:2] = flat
            xb[s, hb, :, 3:3 + 2 * L:2] = ext[:, 1:L + 1]

    # w: [C, (dh,dw), (hb,t,r), (wp,q)] -> [t, hb, c, dh, r, wp, dw, q]
    wb = (
        wc.reshape(C, KW, KW, 2, T, Rh, W // 2, 2)
        .transpose(4, 3, 0, 1, 5, 6, 2, 7)
        .copy()
    )  # [T, hb, C, dh, r, wp, dw, q]
    # width-edge taps multiply zero padding in the reference -> zero them
    wb[:, :, :, :, :, 0, 0, 0] = 0
    wb[:, :, :, :, :, W // 2 - 1, KW - 1, 1] = 0
    return (
        xb.reshape(NXT, 128, X3F),
        np.ascontiguousarray(wb.reshape(T, 128, WF)),
    )


def _make_in_maps(x: np.ndarray, conv_weights: np.ndarray):
    x = np.asarray(x, dtype=np.float32).astype(BF16)
    w5 = (
        np.asarray(conv_weights, dtype=np.float32)
        .astype(BF16)
        .reshape(N, C, KW * KW, H, W)
    )
    xh = np.zeros((N, C, H + 2, W), dtype=BF16)
    xh[:, :, 1:-1, :] = x

    in_maps = []
    for i in range(NCORES):
        n, hf = divmod(i, 2)
        xb, wb = _pack_core(xh[n], w5[n], hf)
        in_maps.append({"x": xb, "w": wb})
    return in_maps


def kernel(x: np.ndarray, conv_weights: np.ndarray) -> np.ndarray:
    nc = _get_nc()
    in_maps = _make_in_maps(x, conv_weights)
    res = run_bass_kernel_spmd(nc, in_maps, list(range(NCORES)))
    out = np.empty((N, C, H, W), dtype=np.float32)
    for i in range(NCORES):
        n, hf = divmod(i, 2)
        yb = np.asarray(res.results[i]["y"]).reshape(T, 2, C, Rh, W)
        oc = yb.transpose(2, 1, 0, 3, 4).reshape(C, HH, W).astype(np.float32)
        out[n, :, hf * HH:(hf + 1) * HH, :] = oc
    return out


# revision 5
# speedup vs baseline: 1.5215x; 1.0496x over previous
"""Dynamic depthwise 3x3 conv (per-pixel weights) on 8 Trainium2 NeuronCores.

Problem:
  x:            [4, 64, 256, 256]  f32
  conv_weights: [4, 576, 256, 256] f32  (= [4, 64ch * 9tap, 256, 256])
  out[n,c,h,w] = sum_k w[n, c*9+k, h, w] * xpad[n, c, h+ki, w+kj],  k=(ki,kj)

Sharding: pure data parallel over (batch n, H-half) -> 8 shards.

This version moves all streams to bf16 (halves HBM traffic; rel err ~0.5%
vs the 2e-2 gate) and replaces the 1-elem/cycle segmented MAC with a custom
2x-mode DVE op (SEG_MAC2_ANT) that retires 2 bf16 MACs/cycle:

  * outputs are processed in adjacent pairs (j, j+1); each DVE cycle reads a
    packed bf16 pair of weights (one tap for both outputs) and a packed pair
    of x values
  * x is stored in an "overlapped pairs" layout x3[2e]=x[e], x3[2e+1]=x[e+1],
    which turns the overlapping 3-tap windows into contiguous aligned reads
  * two independent accumulators (even/odd j) live on different ALU blocks;
    subdim pages of 6 elements = 3 cycles per 3-tap dot; writes are gated to
    the page-final cycle (out_last_subdim_enable), one packed write per pair
  * the instruction carries perf_max=1 in byte-36[7:6] (the firmware decodes
    it; bass.py never sets it, so a scoped monkeypatch injects it)

Per tile (J=2048 outputs/partition): 3 per-dh segmacs (3 cycles/pair each)
+ 2 bf16 tensor_adds combine the dh partials. Weight-edge taps (wd=0,dw=0 /
wd=255,dw=2) are zeroed host-side so width wrap-around contributes nothing.
"""

import sys

sys.path.insert(0, "/opt/trn_rl_repo")

import numpy as np
import ml_dtypes

import concourse.bass as bass
import concourse.bacc as bacc
import concourse.tile as tile
from concourse import mybir
from concourse.bass_utils import run_bass_kernel_spmd

import concourse.dve_ops as dve_ops
import concourse.bass_isa as bass_isa
from concourse.dve_spec import Spec, Src0, Src1
from concourse.dve_uop import (
    ENABLE,
    AluInp,
    AluOp,
    DelayInp,
    DveOpSpec,
    InpSel,
    OutPath,
    OutSel,
    Trigger,
    UopConfig,
    UopDpConfig,
)

# ---------------------------------------------------------------------------
# SEG_MAC2_ANT: j-paired segmented 3-tap MAC, 2 bf16 MACs/cycle.
#   out[p, 2*jp + q] = sum_dw w[p, 6*jp + 2*dw + q] * x3[p, 4*jp + 2*dw + q]
# ---------------------------------------------------------------------------

OP_NAME = "SEG_MAC2_ANT"

_SRC = Trigger.SRC_TENSOR_DONE
_SUB = Trigger.SUB_DIM_DONE
_CNT = Trigger.COUNT
_NON = Trigger.NONE


def _dp(mode):
    """mode: 'seed' (acc<-0), 'steady' (acc+=p), 'step' (acc=0+p).
    Chains: c0=SRC_0 then p_e, c1=SRC_1 then p_o, c2=SRC_0_HI then acc_e,
    c3=SRC_1_HI, c4=ZERO. acc_e at block b2, acc_o at block b7."""
    dp = [UopDpConfig() for _ in range(8)]
    for st in range(8):
        dp[st].pass_through_delay(0, 1, 2, 3, 4)
    dp[0].enable_alu(AluOp.MULTIPLY, AluInp.PREV_DELAY_0, AluInp.PREV_DELAY_1)
    dp[1].enable_alu(AluOp.MULTIPLY, AluInp.PREV_DELAY_2, AluInp.PREV_DELAY_3)
    dp[1].enable_delay_from_src(DelayInp.PREV_ALU_OUT, 0)  # capture p_e
    if mode == "seed":
        dp[2].enable_alu(AluOp.BYPASS, AluInp.PREV_DELAY_4, AluInp.PREV_DELAY_4)
    elif mode == "steady":
        dp[2].enable_alu(AluOp.ADD, AluInp.CURR_ALU_OUT, AluInp.PREV_DELAY_0)
    else:
        dp[2].enable_alu(AluOp.ADD, AluInp.PREV_DELAY_4, AluInp.PREV_DELAY_0)
    dp[2].enable_delay_from_src(DelayInp.PREV_ALU_OUT, 1)  # capture p_o
    dp[3].enable_delay_from_src(DelayInp.PREV_ALU_OUT, 2)  # capture acc_e
    for st in range(3, 7):
        dp[st].pass_through_alu()
    if mode == "seed":
        dp[7].enable_alu(AluOp.BYPASS, AluInp.PREV_DELAY_4, AluInp.PREV_DELAY_4)
    elif mode == "steady":
        dp[7].enable_alu(AluOp.ADD, AluInp.CURR_ALU_OUT, AluInp.PREV_DELAY_1)
    else:
        dp[7].enable_alu(AluOp.ADD, AluInp.PREV_DELAY_4, AluInp.PREV_DELAY_1)
    return dp


def _uop(mode, trig, nxt, repeat, consume, write):
    u = UopConfig(datapath_config=_dp(mode))
    u.enable_input(InpSel.SRC_0, 1)
    u.enable_input(InpSel.SRC_1, 2)
    u.enable_input(InpSel.SRC_0_HI, 3)
    u.enable_input(InpSel.SRC_1_HI, 4)
    u.enable_input(InpSel.ZERO, 5)
    if write:
        u.enable_output(OutSel.DELAY_2, OutPath.WR0_LO)   # acc_e
        u.enable_output(OutSel.ALU_OUT, OutPath.WR0_HI)   # acc_o (b7)
        u.out_last_subdim_enable = ENABLE
    if consume:
        u.require_inp0 = ENABLE
        u.require_inp1 = ENABLE
    u.repeat_count = repeat
    u.trigger = trig
    u.next_uop = nxt
    return u


def _program():
    return [
        _uop("seed", (_CNT, _NON, _NON), (1, 0, 0), 1, False, False),
        _uop("steady", (_SRC, _SUB, _NON), (0, 2, 0), 0, True, True),
        _uop("step", (_SRC, _SUB, _CNT), (0, 2, 1), 1, True, True),
    ]


def _segmac2_ref(in0, in1, c0, c1, c2):
    P = in0.shape[0]
    a = np.asarray(in0, np.float32).reshape(P, -1, 3, 2)
    b = np.asarray(in1, np.float32).reshape(P, -1, 3, 2)
    return (a * b).sum(axis=2)


def get_segmac2_op():
    existing = getattr(dve_ops, "_ANT_SEG_MAC2", None)
    if existing is not None:
        return existing

    spec = Spec(body=Src0 * Src1, reference=_segmac2_ref)
    op = dve_ops.DveOp(OP_NAME, spec, subdim=True, uops_sha={})
    dve_ops.OPS.append(op)
    row = dve_ops._CUSTOM_DVE_ROW_BASE + len(dve_ops.OPS) - 1
    assert row < 0x20
    dve_ops._SUB_OPCODE_FOR_NAME[OP_NAME] = row
    dve_ops.CUSTOM_DVE_SPECS[OP_NAME] = spec

    compiled = DveOpSpec(
        name=OP_NAME,
        opcode=row,
        uops=_program(),
        uops_2x=_program(),
        rd1_en=True,
        perf_max=1,
    )
    compiled.validate("v3")
    dve_ops._COMPILE_CACHE[(OP_NAME, "v3")] = compiled

    import concourse.bass as bass_mod

    orig = bass_isa.InstCustomDveAnt

    def patched(*args, **kwargs):
        if kwargs.get("op_name") == OP_NAME:
            kwargs.setdefault("perf_max", 1)
        return orig(*args, **kwargs)

    bass_isa.InstCustomDveAnt = patched
    bass_mod.bass_isa.InstCustomDveAnt = patched

    dve_ops._ANT_SEG_MAC2 = op
    return op


def window_ap(sl, dims):
    """AP over `sl`'s tensor/offset with explicit free dims [[step, count],...]."""
    import bass_rust

    return bass_rust.AP(
        sl.tensor,
        sl.offset,
        [list(sl.ap[0])] + [list(d) for d in dims],
        sl.const_val,
        sl.runtime_checks,
        sl.dep_tracking_offset,
    )


# ---------------------------------------------------------------------------
# Kernel
# ---------------------------------------------------------------------------

N, C, H, W = 4, 64, 256, 256
KW = 3
NCORES = 8
HH = H // 2          # rows per core
RB = HH // 2         # rows per partition block (64)
Rh = 8               # rows per h-tile
T = RB // Rh         # h-tiles per core (8)
J = Rh * W           # outputs per partition per tile (2048)
JP = J // 2          # output pairs (1024)
WSEG = 6 * JP        # w elems per dh chunk
WF = 3 * WSEG        # w elems per tile (9*J)
NXT = 4              # resident x tiles per core
XB = RB // NXT       # output rows per x tile (16)
XR = XB + 2          # rows per x tile incl halo (18)
X3F = 2 * XR * W + 4  # doubled x elems per x tile (+guards)
BF = mybir.dt.bfloat16
BF16 = ml_dtypes.bfloat16

_CACHE = {}


def _build():
    op = get_segmac2_op()
    nc = bacc.Bacc("TRN2", target_bir_lowering=False, debug=False, num_devices=NCORES)
    x_in = nc.dram_tensor("x", [NXT, 128, X3F], BF, kind="ExternalInput")
    w_in = nc.dram_tensor("w", [T, KW, 128, WSEG], BF, kind="ExternalInput")
    y_out = nc.dram_tensor("y", [T, 128, J], BF, kind="ExternalOutput")

    with tile.TileContext(nc) as tc:
        with (
            tc.tile_pool(name="xp", bufs=1) as xpool,
            tc.tile_pool(name="wp", bufs=2) as wpool,
            tc.tile_pool(name="o0", bufs=2) as o0pool,
            tc.tile_pool(name="pa", bufs=1) as papool,
            tc.tile_pool(name="pb", bufs=1) as pbpool,
        ):
            xtiles = []
            for s in range(NXT):
                xt = xpool.tile([128, X3F], BF, tag=f"x{s}")
                nc.scalar.dma_start(out=xt[:], in_=x_in[s])
                xtiles.append(xt)

            for t in range(T):
                wt = wpool.tile([128, WF], BF)
                for dh in range(KW):
                    # alternate the two HWDGE rings so both descriptor
                    # streams stay busy
                    eng = nc.sync if (t * KW + dh) % 2 == 0 else nc.scalar
                    eng.dma_start(
                        out=wt[:, dh * WSEG:(dh + 1) * WSEG],
                        in_=w_in[t, dh],
                    )

                s = t * Rh // XB
                rb = t * Rh - s * XB
                xt = xtiles[s]

                ot = o0pool.tile([128, J], BF)
                pa = papool.tile([128, J], BF)
                pb = pbpool.tile([128, J], BF)
                for dh, tgt in ((0, ot), (1, pa), (2, pb)):
                    xbase = 2 * (rb + dh) * W
                    nc.vector._custom_dve(
                        op,
                        out=window_ap(tgt[:, 0:J], [[2, JP], [1, 2]]),
                        in0=window_ap(
                            wt[:, dh * WSEG:(dh + 1) * WSEG], [[6, JP], [1, 6]]
                        ),
                        in1=window_ap(
                            xt[:, xbase:xbase + 4 * JP + 2], [[4, JP], [1, 6]]
                        ),
                    )
                nc.vector.tensor_add(ot[:], ot[:], pa[:])
                nc.vector.tensor_add(ot[:], ot[:], pb[:])

                nc.gpsimd.dma_start(out=y_out[t], in_=ot[:])
    nc.compile()
    return nc


def _get_nc():
    if "nc" not in _CACHE:
        _CACHE["nc"] = _build()
    return _CACHE["nc"]


def _pack_core(xh_n: np.ndarray, w5_n: np.ndarray, hf: int):
    """Repack one core's shard (bf16 inputs).

    xh_n: [C, H+2, W] H-padded x for batch n (bf16); w5_n: [C, 9, H, W] bf16.
    Returns x_blocks [NXT, 128, X3F], w_blocks [T, 128, WF].
    """
    xc = xh_n[:, hf * HH:hf * HH + HH + 2, :]          # [C, HH+2, W]
    wc = w5_n[:, :, hf * HH:(hf + 1) * HH, :]          # [C, 9, HH, W]

    # x3: doubled layout per x tile, per partition block
    xb = np.zeros((NXT, 2, C, X3F), dtype=BF16)
    L = XR * W
    for s in range(NXT):
        for hb in range(2):
            r0 = hb * RB + s * XB
            flat = xc[:, r0:r0 + XR, :].reshape(C, L)
            ext = np.zeros((C, L + 2), dtype=BF16)
            ext[:, :L] = flat
            if r0 + XR < HH + 2:
                ext[:, L:L + 2] = xc[:, r0 + XR, :2]
            xb[s, hb, :, 2:2 + 2 * L:2] = flat
            xb[s, hb, :, 3:3 + 2 * L:2] = ext[:, 1:L + 1]

    # w: [C, (dh,dw), (hb,t,r), (wp,q)] -> [t, dh, hb, c, r, wp, dw, q]
    # (dh-major so each per-dh DMA chunk is fully contiguous in HBM)
    wb = (
        wc.reshape(C, KW, KW, 2, T, Rh, W // 2, 2)
        .transpose(4, 1, 3, 0, 5, 6, 2, 7)
        .copy()
    )  # [T, dh, hb, C, r, wp, dw, q]
    # width-edge taps multiply zero padding in the reference -> zero them
    wb[:, :, :, :, :, 0, 0, 0] = 0
    wb[:, :, :, :, :, W // 2 - 1, KW - 1, 1] = 0
    return (
        xb.reshape(NXT, 128, X3F),
        np.ascontiguousarray(wb.reshape(T, KW, 128, WSEG)),
    )


def _make_in_maps(x: np.ndarray, conv_weights: np.ndarray):
    x = np.asarray(x, dtype=np.float32).astype(BF16)
    w5 = (
        np.asarray(conv_weights, dtype=np.float32)
        .astype(BF16)
        .reshape(N, C, KW * KW, H, W)
    )
    xh = np.zeros((N, C, H + 2, W), dtype=BF16)
    xh[:, :, 1:-1, :] = x

    in_maps = []
    for i in range(NCORES):
        n, hf = divmod(i, 2)
        xb, wb = _pack_core(xh[n], w5[n], hf)
        in_maps.append({"x": xb, "w": wb})
    return in_maps


def kernel(x: np.ndarray, conv_weights: np.ndarray) -> np.ndarray:
    nc = _get_nc()
    in_maps = _make_in_maps(x, conv_weights)
    res = run_bass_kernel_spmd(nc, in_maps, list(range(NCORES)))
    out = np.empty((N, C, H, W), dtype=np.float32)
    for i in range(NCORES):
        n, hf = divmod(i, 2)
        yb = np.asarray(res.results[i]["y"]).reshape(T, 2, C, Rh, W)
        oc = yb.transpose(2, 1, 0, 3, 4).reshape(C, HH, W).astype(np.float32)
        out[n, :, hf * HH:(hf + 1) * HH, :] = oc
    return out
